# revision 20
# baseline (speedup 1.0000x reference)
import os
import sys
sys.path.insert(0, '/opt/trn_rl_repo')
import numpy as np
import ml_dtypes

BF16 = ml_dtypes.bfloat16

P = 128
NCORES = 8
F = 128
NTAB = 4          # gather tables (int16 index range: rows per table <= 32767)
G_BLK = int(os.environ.get("G_BLK", "4"))       # dest blocks per gather group
S_MODE = os.environ.get("S_MODE", "build")      # build | stream


def _build_bass(nblk, n_pad, ncols, idxcols, ch_max, ops_plan, weights, biases, Wp, bp):
    """SPMD Bass program. Identical trace on all cores; per-core data via
    ExternalInputs.

    The GCN normalization is folded: gather tables hold h*dis[node] (source
    side); the dest-side dis is applied inside each layer's bias/relu and,
    for the final conv, once in the epilogue. The scatter one-hot S is
    therefore pure 0/1.

    ops_plan: list of groups; each group is (ops, blocks) with
        op = (t, chunk_off, nchunks)            # one dma_gather per src range
        block = (j, [(t, chunk_off, k0, nk)])   # matmul chunks per block
    """
    from concourse import bass, bacc, mybir
    import concourse.tile as tile

    slice_rows = nblk * P
    T = n_pad // NTAB

    nc = bacc.Bacc(num_devices=NCORES, num_swdge_queues=4)

    bf = mybir.dt.bfloat16
    f32 = mybir.dt.float32

    x_in = nc.declare_dram_parameter("x_in", [slice_rows, F], bf, isOutput=False)
    idx_in = nc.declare_dram_parameter("idx_in", [P, idxcols], mybir.dt.int16, isOutput=False)
    disb_in = nc.declare_dram_parameter("disb", [P, nblk], f32, isOutput=False)
    disr_in = nc.declare_dram_parameter("disr", [1, slice_rows], f32, isOutput=False)
    if S_MODE == "stream":
        sf_in = nc.declare_dram_parameter("sf_in", [P, ncols * P], bf, isOutput=False)
    else:
        mdlc_in = nc.declare_dram_parameter("mdlc", [P, ncols], f32, isOutput=False)
    y_out = nc.declare_dram_parameter("y_out", [2, slice_rows], f32, isOutput=True)

    # internal DRAM
    ag_in_x = nc.dram_tensor("ag_in_x", [slice_rows, F], bf)
    h_slice = [nc.dram_tensor(f"h_slice{i}", [slice_rows, F], bf) for i in range(3)]
    v_full = [nc.dram_tensor(f"v_full{i}", [n_pad, F], bf, addr_space="Shared")
              for i in range(4)]

    # inline constants (same on every core)
    W_d = [nc.inline_tensor(np.ascontiguousarray(w.astype(BF16)), name=f"W{i}")
           for i, w in enumerate(weights)]
    B_d = [nc.inline_tensor(np.broadcast_to(b.astype(np.float32), (P, F)).copy(), name=f"B{i}")
           for i, b in enumerate(biases)]
    Wp_d = [nc.inline_tensor(np.ascontiguousarray(Wp[i*F:(i+1)*F, :].astype(BF16)), name=f"Wp{i}")
            for i in range(3)]
    bpd = float(bp[0] - bp[1])
    if S_MODE == "build":
        iota_np = np.broadcast_to(np.arange(P, dtype=np.float32), (P, P)).astype(BF16)
        iota_d = nc.inline_tensor(np.ascontiguousarray(iota_np), name="iota_c")

    AF = mybir.ActivationFunctionType
    ALU = mybir.AluOpType
    rg = [list(range(NCORES))]

    with tile.TileContext(nc) as tc:
        with (
            tc.tile_pool(name="const", bufs=1) as cpool,
            tc.tile_pool(name="msg", bufs=6) as mpool,
            tc.tile_pool(name="sS", bufs=6) as spool,
            tc.tile_pool(name="gts", bufs=4) as gpool,
            tc.tile_pool(name="hout", bufs=4) as hpool,
            tc.tile_pool(name="epi", bufs=1) as epool,
            tc.tile_pool(name="psum", bufs=3, space="PSUM") as psum,
            tc.tile_pool(name="psum2", bufs=2, space="PSUM") as psum2,
        ):
            nc.sync.dma_start(out=ag_in_x[:, :], in_=x_in[:, :])
            nc.gpsimd.collective_compute(
                "AllGather", ALU.bypass, replica_groups=rg,
                ins=[ag_in_x[:].opt()], outs=[v_full[0][:].opt()],
            )

            W_t, B_t, Wp_t = [], [], []
            for i in range(3):
                wt = cpool.tile([P, F], bf, tag=f"w{i}")
                nc.sync.dma_start(out=wt[:], in_=W_d[i][:, :])
                W_t.append(wt)
                bt = cpool.tile([P, F], f32, tag=f"b{i}")
                nc.sync.dma_start(out=bt[:], in_=B_d[i][:, :])
                B_t.append(bt)
                wpt = cpool.tile([P, 2], bf, tag=f"wp{i}")
                nc.sync.dma_start(out=wpt[:], in_=Wp_d[i][:, :])
                Wp_t.append(wpt)

            idx_t = cpool.tile([P, idxcols], mybir.dt.int16)
            nc.sync.dma_start(out=idx_t[:], in_=idx_in[:, :])
            disb_t = cpool.tile([P, nblk], f32)
            nc.sync.dma_start(out=disb_t[:], in_=disb_in[:, :])
            if S_MODE == "build":
                iota_t = cpool.tile([P, P], bf)
                nc.sync.dma_start(out=iota_t[:], in_=iota_d[:, :])
                mdlc_t = cpool.tile([P, ncols], f32)
                nc.sync.dma_start(out=mdlc_t[:], in_=mdlc_in[:, :])

            yT_acc = cpool.tile([2, slice_rows], f32)
            nc.vector.memset(yT_acc[:], 0.0)

            qrot = 0
            # ---- 4 propagations ----
            for i in range(4):
                src = v_full[i]
                for g_ops, g_blocks in ops_plan:
                    msgs = {}
                    ss = {}
                    for (t, chunk_off, nchunks) in g_ops:
                        mt = mpool.tile([P, ch_max, F], bf, tag="msg")
                        n = nchunks * P
                        nc.gpsimd.dma_gather(
                            mt[:, :nchunks, :],
                            src[t*T:(t+1)*T, :],
                            idx_t[:, 8*chunk_off: 8*chunk_off + 8*nchunks],
                            n, n, F,
                            single_packet=False,
                            queue_num=qrot,
                        )
                        qrot = (qrot + 1) % 4
                        msgs[t] = (mt, chunk_off)
                        if S_MODE == "stream":
                            st = spool.tile([P, ch_max * P], bf, tag="S")
                            nc.sync.dma_start(
                                out=st[:, :nchunks*P],
                                in_=sf_in[:, chunk_off*P:(chunk_off+nchunks)*P])
                            ss[t] = st
                    for (j, segs) in g_blocks:
                        nk_tot = sum(nk for (_, _, _, nk) in segs)
                        gt = psum.tile([P, P], f32, tag="gt", space="PSUM")
                        kk = 0
                        for (t, chunk_off, k0, nk) in segs:
                            mt, op_off = msgs[t]
                            for k in range(nk):
                                col = chunk_off + k0 + k
                                cl = col - op_off
                                if S_MODE == "stream":
                                    S = ss[t][:, cl*P:(cl+1)*P]
                                else:
                                    St = spool.tile([P, P], bf, tag="S")
                                    nc.vector.tensor_scalar(
                                        out=St[:], in0=iota_t[:],
                                        scalar1=mdlc_t[:, col:col+1], scalar2=None,
                                        op0=ALU.is_equal)
                                    S = St[:]
                                nc.tensor.matmul(
                                    out=gt[:], lhsT=mt[:, cl, :], rhs=S,
                                    start=(kk == 0), stop=(kk == nk_tot - 1))
                                kk += 1
                        gts = gpool.tile([P, P], bf, tag="gts")
                        nc.scalar.copy(out=gts[:], in_=gt[:])
                        if i < 3:
                            hp = psum2.tile([P, P], f32, tag="hp", space="PSUM")
                            nc.tensor.matmul(out=hp[:], lhsT=gts[:], rhs=W_t[i][:],
                                             start=True, stop=True)
                            hb = hpool.tile([P, P], f32, tag="hb")
                            # h = relu(dis_dst*hp + B); table row = h*dis (src fold)
                            nc.vector.scalar_tensor_tensor(
                                out=hb[:], in0=hp[:], scalar=disb_t[:, j:j+1],
                                in1=B_t[i][:], op0=ALU.mult, op1=ALU.add)
                            hb2 = hpool.tile([P, P], bf, tag="hb2")
                            nc.vector.tensor_scalar(
                                out=hb2[:], in0=hb[:],
                                scalar1=0.0, op0=ALU.max,
                                scalar2=disb_t[:, j:j+1], op1=ALU.mult)
                            nc.sync.dma_start(out=h_slice[i][j*P:(j+1)*P, :], in_=hb2[:])
                        if i >= 1:
                            yp = psum2.tile([2, P], f32, tag="yp", space="PSUM")
                            nc.tensor.matmul(out=yp[:], lhsT=Wp_t[i-1][:], rhs=gts[:],
                                             start=True, stop=True)
                            nc.vector.tensor_tensor(
                                out=yT_acc[:, j*P:(j+1)*P],
                                in0=yT_acc[:, j*P:(j+1)*P], in1=yp[:], op=ALU.add)
                if i < 3:
                    nc.gpsimd.collective_compute(
                        "AllGather", ALU.bypass, replica_groups=rg,
                        ins=[h_slice[i][:].opt()], outs=[v_full[i+1][:].opt()],
                    )

            # ---- epilogue: d = dis*(y0-y1); p0 = sigmoid(d + bp0-bp1) ----
            nchunk_e = 8
            cw = slice_rows // nchunk_e
            for ce in range(nchunk_e):
                cs = ce * cw
                t1 = epool.tile([1, cw], f32, tag="t1")
                nc.sync.dma_start(out=t1[:], in_=yT_acc[1:2, cs:cs+cw])
                dr = epool.tile([1, cw], f32, tag="dr")
                nc.sync.dma_start(out=dr[:], in_=disr_in[0:1, cs:cs+cw])
                dif = epool.tile([1, cw], f32, tag="dif")
                nc.vector.tensor_tensor(out=dif[:], in0=yT_acc[0:1, cs:cs+cw],
                                        in1=t1[:], op=ALU.subtract)
                nc.vector.tensor_tensor(out=dif[:], in0=dif[:], in1=dr[:], op=ALU.mult)
                nc.vector.tensor_scalar(out=dif[:], in0=dif[:],
                                        scalar1=bpd, scalar2=None, op0=ALU.add)
                sig = epool.tile([1, cw], f32, tag="sig")
                nc.scalar.activation(out=sig[:], in_=dif[:], func=AF.Sigmoid)
                om = epool.tile([1, cw], f32, tag="om")
                nc.vector.tensor_scalar(out=om[:], in0=sig[:],
                                        scalar1=-1.0, op0=ALU.mult,
                                        scalar2=1.0, op1=ALU.add)
                nc.sync.dma_start(out=y_out[0:1, cs:cs+cw], in_=sig[:])
                nc.sync.dma_start(out=y_out[1:2, cs:cs+cw], in_=om[:])

    nc.compile()
    return nc


def _prep(x, edge_index, n_pad, nblk):
    """Host-side prep: fold norm into tables, bucket edges by
    (dest block, src range), pad buckets to the max across cores, build
    gather indices, prop-0 pre-gathered messages, and (for S_MODE=stream)
    the streamed one-hot scatter matrices."""
    N = x.shape[0]
    T = n_pad // NTAB
    slice_rows = nblk * P
    nblk_tot = NCORES * nblk

    row = np.concatenate([edge_index[0].astype(np.int64), np.arange(N, dtype=np.int64)])
    col = np.concatenate([edge_index[1].astype(np.int64), np.arange(N, dtype=np.int64)])
    deg = np.bincount(col, minlength=N).astype(np.float32)
    dis = np.where(deg > 0, 1.0 / np.sqrt(deg), 0.0).astype(np.float32)

    gblk = col // P
    trng = row // T
    key = gblk * NTAB + trng
    order = np.argsort(key, kind='stable')
    row_s = row[order].astype(np.int32)
    col_s = col[order].astype(np.int32)

    cnt = np.bincount(key, minlength=nblk_tot * NTAB).reshape(NCORES, nblk, NTAB)
    starts = np.zeros(nblk_tot * NTAB + 1, np.int64)
    np.cumsum(cnt.reshape(-1), out=starts[1:])

    K_jt = np.ceil(cnt.max(axis=0) / P).astype(np.int64)   # [nblk, NTAB]

    ops_plan = []
    colp = 0
    ch_max = 0
    for g in range(0, nblk, G_BLK):
        blocks = list(range(g, min(g + G_BLK, nblk)))
        g_ops = []
        segs_per_block = {j: [] for j in blocks}
        for t in range(NTAB):
            nchunks = int(sum(K_jt[j, t] for j in blocks))
            if nchunks == 0:
                continue
            g_ops.append((t, colp, nchunks))
            ch_max = max(ch_max, nchunks)
            k0 = 0
            for j in blocks:
                nk = int(K_jt[j, t])
                if nk:
                    segs_per_block[j].append((t, colp, k0, nk))
                k0 += nk
            colp += nchunks
        g_blocks = [(j, segs_per_block[j]) for j in blocks]
        ops_plan.append((g_ops, g_blocks))
    ncols = colp
    idxcols = 8 * ncols

    xn_pad = np.zeros((n_pad, F), dtype=BF16)
    xn_pad[:N] = (x * dis[:, None]).astype(BF16)   # src-fold
    dis_pad = np.zeros(n_pad, np.float32)
    dis_pad[:N] = dis

    in_maps = []
    for c in range(NCORES):
        idx_flat = np.zeros(ncols * P, np.int16)
        src_flat = np.zeros(ncols * P, np.int64)
        dlc_flat = np.zeros(ncols * P, np.int64)
        val_flat = np.zeros(ncols * P, bool)
        for g_ops, g_blocks in ops_plan:
            for (t, chunk_off, nchunks) in g_ops:
                p0 = chunk_off * P
                for (j, segs) in g_blocks:
                    for (tt, _, k0, nk) in segs:
                        if tt != t:
                            continue
                        s = starts[((c * nblk + j) * NTAB) + t]
                        e = starts[((c * nblk + j) * NTAB) + t + 1]
                        n = e - s
                        sl = slice(p0 + k0 * P, p0 + k0 * P + n)
                        idx_flat[sl] = (row_s[s:e] % T).astype(np.int16)
                        src_flat[sl] = row_s[s:e]
                        dlc_flat[sl] = col_s[s:e] - (c * nblk + j) * P
                        val_flat[sl] = True
        idx_arr = np.tile(idx_flat.reshape(-1, 16).T, (8, 1)).astype(np.int16)
        im = {
            "x_in": np.ascontiguousarray(xn_pad[c*slice_rows:(c+1)*slice_rows]),
            "idx_in": np.ascontiguousarray(idx_arr),
            "disb": np.ascontiguousarray(
                dis_pad[c*slice_rows:(c+1)*slice_rows].reshape(nblk, P).T),
            "disr": np.ascontiguousarray(
                dis_pad[c*slice_rows:(c+1)*slice_rows].reshape(1, slice_rows)),
        }
        if S_MODE == "stream":
            # sf[p, c*128+q] = 1 if dlc[c*128+p] == q and valid
            oh = (dlc_flat.reshape(ncols, P)[:, :, None] ==
                  np.arange(P)[None, None, :])
            oh &= val_flat.reshape(ncols, P)[:, :, None]
            sf = np.ascontiguousarray(
                oh.transpose(1, 0, 2).reshape(P, ncols * P).astype(BF16))
            im["sf_in"] = sf
        else:
            dlc_m = np.where(val_flat, dlc_flat, -1).astype(np.float32)
            im["mdlc"] = np.ascontiguousarray(dlc_m.reshape(ncols, P).T)
        in_maps.append(im)
    return ops_plan, in_maps, ncols, idxcols, ch_max


LAST_RESULTS = None
LAST_NC = None
LAST_IN_MAPS = None


def kernel(x, edge_index, W0, b0, W1, b1, W2, b2, Wp, bp):
    global LAST_RESULTS, LAST_NC, LAST_IN_MAPS
    from concourse.bass_utils import run_bass_kernel_spmd

    x = np.asarray(x, dtype=np.float32)
    edge_index = np.asarray(edge_index)
    N = x.shape[0]
    nblk = int(np.ceil(N / (NCORES * P)))
    n_pad = NCORES * nblk * P
    slice_rows = nblk * P

    ops_plan, in_maps, ncols, idxcols, ch_max = _prep(x, edge_index, n_pad, nblk)

    nc = _build_bass(
        nblk, n_pad, ncols, idxcols, ch_max, ops_plan,
        [np.asarray(W0), np.asarray(W1), np.asarray(W2)],
        [np.asarray(b0), np.asarray(b1), np.asarray(b2)],
        np.asarray(Wp), np.asarray(bp),
    )

    res = run_bass_kernel_spmd(nc, in_maps, list(range(NCORES)))
    LAST_RESULTS = res
    LAST_NC = nc
    LAST_IN_MAPS = in_maps

    out = np.zeros((n_pad, 2), np.float32)
    for c in range(NCORES):
        yT = res.results[c]["y_out"]          # [2, slice_rows]
        out[c*slice_rows:(c+1)*slice_rows] = yT.T
    return out[:N]


# revision 21
# speedup vs baseline: 1.0347x; 1.0347x over previous
import os
import sys
sys.path.insert(0, '/opt/trn_rl_repo')
import numpy as np
import ml_dtypes

BF16 = ml_dtypes.bfloat16

P = 128
NCORES = 8
F = 128
NTAB = 4          # gather tables (int16 index range: rows per table <= 32767)
G_BLK = int(os.environ.get("G_BLK", "4"))       # dest blocks per gather group
S_MODE = os.environ.get("S_MODE", "build")      # build | stream


def _build_bass(nblk, n_pad, ncols, idxcols, ch_max, ops_plan, weights, biases, Wp, bp):
    """SPMD Bass program. Identical trace on all cores; per-core data via
    ExternalInputs.

    The GCN normalization is folded: gather tables hold h*dis[node] (source
    side); the dest-side dis is applied inside each layer's bias/relu and,
    for the final conv, once in the epilogue. The scatter one-hot S is
    therefore pure 0/1.

    ops_plan: list of groups; each group is (ops, blocks) with
        op = (t, chunk_off, nchunks)            # one dma_gather per src range
        block = (j, [(t, chunk_off, k0, nk)])   # matmul chunks per block
    """
    from concourse import bass, bacc, mybir
    import concourse.tile as tile

    slice_rows = nblk * P
    T = n_pad // NTAB

    nc = bacc.Bacc(num_devices=NCORES, num_swdge_queues=4)

    bf = mybir.dt.bfloat16
    f32 = mybir.dt.float32

    xg_in = nc.declare_dram_parameter("xg_in", [ncols * P, F], bf, isOutput=False)
    idx_in = nc.declare_dram_parameter("idx_in", [P, idxcols], mybir.dt.int16, isOutput=False)
    disb_in = nc.declare_dram_parameter("disb", [P, nblk], f32, isOutput=False)
    disr_in = nc.declare_dram_parameter("disr", [1, slice_rows], f32, isOutput=False)
    if S_MODE == "stream":
        sf_in = nc.declare_dram_parameter("sf_in", [P, ncols * P], bf, isOutput=False)
    else:
        mdlc_in = nc.declare_dram_parameter("mdlc", [P, ncols], f32, isOutput=False)
    y_out = nc.declare_dram_parameter("y_out", [2, slice_rows], f32, isOutput=True)

    # internal DRAM
    h_slice = [nc.dram_tensor(f"h_slice{i}", [slice_rows, F], bf) for i in range(3)]
    v_full = [nc.dram_tensor(f"v_full{i}", [n_pad, F], bf, addr_space="Shared")
              for i in range(3)]

    # inline constants (same on every core)
    W_d = [nc.inline_tensor(np.ascontiguousarray(w.astype(BF16)), name=f"W{i}")
           for i, w in enumerate(weights)]
    B_d = [nc.inline_tensor(np.broadcast_to(b.astype(np.float32), (P, F)).copy(), name=f"B{i}")
           for i, b in enumerate(biases)]
    Wp_d = [nc.inline_tensor(np.ascontiguousarray(Wp[i*F:(i+1)*F, :].astype(BF16)), name=f"Wp{i}")
            for i in range(3)]
    bpd = float(bp[0] - bp[1])
    if S_MODE == "build":
        iota_np = np.broadcast_to(np.arange(P, dtype=np.float32), (P, P)).astype(BF16)
        iota_d = nc.inline_tensor(np.ascontiguousarray(iota_np), name="iota_c")

    AF = mybir.ActivationFunctionType
    ALU = mybir.AluOpType
    rg = [list(range(NCORES))]

    with tile.TileContext(nc) as tc:
        with (
            tc.tile_pool(name="const", bufs=1) as cpool,
            tc.tile_pool(name="msg", bufs=6) as mpool,
            tc.tile_pool(name="sS", bufs=6) as spool,
            tc.tile_pool(name="gts", bufs=4) as gpool,
            tc.tile_pool(name="hout", bufs=4) as hpool,
            tc.tile_pool(name="epi", bufs=1) as epool,
            tc.tile_pool(name="psum", bufs=3, space="PSUM") as psum,
            tc.tile_pool(name="psum2", bufs=2, space="PSUM") as psum2,
        ):
            W_t, B_t, Wp_t = [], [], []
            for i in range(3):
                wt = cpool.tile([P, F], bf, tag=f"w{i}")
                nc.sync.dma_start(out=wt[:], in_=W_d[i][:, :])
                W_t.append(wt)
                bt = cpool.tile([P, F], f32, tag=f"b{i}")
                nc.sync.dma_start(out=bt[:], in_=B_d[i][:, :])
                B_t.append(bt)
                wpt = cpool.tile([P, 2], bf, tag=f"wp{i}")
                nc.sync.dma_start(out=wpt[:], in_=Wp_d[i][:, :])
                Wp_t.append(wpt)

            idx_t = cpool.tile([P, idxcols], mybir.dt.int16)
            nc.sync.dma_start(out=idx_t[:], in_=idx_in[:, :])
            disb_t = cpool.tile([P, nblk], f32)
            nc.sync.dma_start(out=disb_t[:], in_=disb_in[:, :])
            if S_MODE == "build":
                iota_t = cpool.tile([P, P], bf)
                nc.sync.dma_start(out=iota_t[:], in_=iota_d[:, :])
                mdlc_t = cpool.tile([P, ncols], f32)
                nc.sync.dma_start(out=mdlc_t[:], in_=mdlc_in[:, :])

            yT_acc = cpool.tile([2, slice_rows], f32)
            nc.vector.memset(yT_acc[:], 0.0)

            qrot = 0
            # ---- 4 propagations ----
            for i in range(4):
                src = v_full[i-1] if i >= 1 else None
                for g_ops, g_blocks in ops_plan:
                    msgs = {}
                    ss = {}
                    for (t, chunk_off, nchunks) in g_ops:
                        mt = mpool.tile([P, ch_max, F], bf, tag="msg")
                        n = nchunks * P
                        if i == 0:
                            nc.sync.dma_start(
                                out=mt[:, :nchunks, :],
                                in_=xg_in[chunk_off*P:(chunk_off+nchunks)*P, :]
                                    .rearrange("(c p) f -> p c f", p=P))
                        else:
                            nc.gpsimd.dma_gather(
                                mt[:, :nchunks, :],
                                src[t*T:(t+1)*T, :],
                                idx_t[:, 8*chunk_off: 8*chunk_off + 8*nchunks],
                                n, n, F,
                                single_packet=False,
                                queue_num=qrot,
                            )
                            qrot = (qrot + 1) % 4
                        msgs[t] = (mt, chunk_off)
                        if S_MODE == "stream":
                            st = spool.tile([P, ch_max * P], bf, tag="S")
                            nc.sync.dma_start(
                                out=st[:, :nchunks*P],
                                in_=sf_in[:, chunk_off*P:(chunk_off+nchunks)*P])
                            ss[t] = st
                    for (j, segs) in g_blocks:
                        nk_tot = sum(nk for (_, _, _, nk) in segs)
                        gt = psum.tile([P, P], f32, tag="gt", space="PSUM")
                        kk = 0
                        for (t, chunk_off, k0, nk) in segs:
                            mt, op_off = msgs[t]
                            for k in range(nk):
                                col = chunk_off + k0 + k
                                cl = col - op_off
                                if S_MODE == "stream":
                                    S = ss[t][:, cl*P:(cl+1)*P]
                                else:
                                    St = spool.tile([P, P], bf, tag="S")
                                    nc.vector.tensor_scalar(
                                        out=St[:], in0=iota_t[:],
                                        scalar1=mdlc_t[:, col:col+1], scalar2=None,
                                        op0=ALU.is_equal)
                                    S = St[:]
                                nc.tensor.matmul(
                                    out=gt[:], lhsT=mt[:, cl, :], rhs=S,
                                    start=(kk == 0), stop=(kk == nk_tot - 1))
                                kk += 1
                        gts = gpool.tile([P, P], bf, tag="gts")
                        nc.scalar.copy(out=gts[:], in_=gt[:])
                        if i < 3:
                            hp = psum2.tile([P, P], f32, tag="hp", space="PSUM")
                            nc.tensor.matmul(out=hp[:], lhsT=gts[:], rhs=W_t[i][:],
                                             start=True, stop=True)
                            hb = hpool.tile([P, P], f32, tag="hb")
                            # h = relu(dis_dst*hp + B); table row = h*dis (src fold)
                            nc.vector.scalar_tensor_tensor(
                                out=hb[:], in0=hp[:], scalar=disb_t[:, j:j+1],
                                in1=B_t[i][:], op0=ALU.mult, op1=ALU.add)
                            hb2 = hpool.tile([P, P], bf, tag="hb2")
                            nc.vector.tensor_scalar(
                                out=hb2[:], in0=hb[:],
                                scalar1=0.0, op0=ALU.max,
                                scalar2=disb_t[:, j:j+1], op1=ALU.mult)
                            nc.sync.dma_start(out=h_slice[i][j*P:(j+1)*P, :], in_=hb2[:])
                        if i >= 1:
                            yp = psum2.tile([2, P], f32, tag="yp", space="PSUM")
                            nc.tensor.matmul(out=yp[:], lhsT=Wp_t[i-1][:], rhs=gts[:],
                                             start=True, stop=True)
                            nc.vector.tensor_tensor(
                                out=yT_acc[:, j*P:(j+1)*P],
                                in0=yT_acc[:, j*P:(j+1)*P], in1=yp[:], op=ALU.add)
                if i < 3:
                    nc.gpsimd.collective_compute(
                        "AllGather", ALU.bypass, replica_groups=rg,
                        ins=[h_slice[i][:].opt()], outs=[v_full[i][:].opt()],
                    )

            # ---- epilogue: d = dis*(y0-y1); p0 = sigmoid(d + bp0-bp1) ----
            nchunk_e = 8
            cw = slice_rows // nchunk_e
            for ce in range(nchunk_e):
                cs = ce * cw
                t1 = epool.tile([1, cw], f32, tag="t1")
                nc.sync.dma_start(out=t1[:], in_=yT_acc[1:2, cs:cs+cw])
                dr = epool.tile([1, cw], f32, tag="dr")
                nc.sync.dma_start(out=dr[:], in_=disr_in[0:1, cs:cs+cw])
                dif = epool.tile([1, cw], f32, tag="dif")
                nc.vector.tensor_tensor(out=dif[:], in0=yT_acc[0:1, cs:cs+cw],
                                        in1=t1[:], op=ALU.subtract)
                nc.vector.tensor_tensor(out=dif[:], in0=dif[:], in1=dr[:], op=ALU.mult)
                nc.vector.tensor_scalar(out=dif[:], in0=dif[:],
                                        scalar1=bpd, scalar2=None, op0=ALU.add)
                sig = epool.tile([1, cw], f32, tag="sig")
                nc.scalar.activation(out=sig[:], in_=dif[:], func=AF.Sigmoid)
                om = epool.tile([1, cw], f32, tag="om")
                nc.vector.tensor_scalar(out=om[:], in0=sig[:],
                                        scalar1=-1.0, op0=ALU.mult,
                                        scalar2=1.0, op1=ALU.add)
                nc.sync.dma_start(out=y_out[0:1, cs:cs+cw], in_=sig[:])
                nc.sync.dma_start(out=y_out[1:2, cs:cs+cw], in_=om[:])

    nc.compile()
    return nc


def _prep(x, edge_index, n_pad, nblk):
    """Host-side prep: fold norm into tables, bucket edges by
    (dest block, src range), pad buckets to the max across cores, build
    gather indices, prop-0 pre-gathered messages, and (for S_MODE=stream)
    the streamed one-hot scatter matrices."""
    N = x.shape[0]
    T = n_pad // NTAB
    slice_rows = nblk * P
    nblk_tot = NCORES * nblk

    row = np.concatenate([edge_index[0].astype(np.int64), np.arange(N, dtype=np.int64)])
    col = np.concatenate([edge_index[1].astype(np.int64), np.arange(N, dtype=np.int64)])
    deg = np.bincount(col, minlength=N).astype(np.float32)
    dis = np.where(deg > 0, 1.0 / np.sqrt(deg), 0.0).astype(np.float32)

    gblk = col // P
    trng = row // T
    key = gblk * NTAB + trng
    order = np.argsort(key, kind='stable')
    row_s = row[order].astype(np.int32)
    col_s = col[order].astype(np.int32)

    cnt = np.bincount(key, minlength=nblk_tot * NTAB).reshape(NCORES, nblk, NTAB)
    starts = np.zeros(nblk_tot * NTAB + 1, np.int64)
    np.cumsum(cnt.reshape(-1), out=starts[1:])

    K_jt = np.ceil(cnt.max(axis=0) / P).astype(np.int64)   # [nblk, NTAB]

    ops_plan = []
    colp = 0
    ch_max = 0
    for g in range(0, nblk, G_BLK):
        blocks = list(range(g, min(g + G_BLK, nblk)))
        g_ops = []
        segs_per_block = {j: [] for j in blocks}
        for t in range(NTAB):
            nchunks = int(sum(K_jt[j, t] for j in blocks))
            if nchunks == 0:
                continue
            g_ops.append((t, colp, nchunks))
            ch_max = max(ch_max, nchunks)
            k0 = 0
            for j in blocks:
                nk = int(K_jt[j, t])
                if nk:
                    segs_per_block[j].append((t, colp, k0, nk))
                k0 += nk
            colp += nchunks
        g_blocks = [(j, segs_per_block[j]) for j in blocks]
        ops_plan.append((g_ops, g_blocks))
    ncols = colp
    idxcols = 8 * ncols

    xn = (x * dis[:, None]).astype(BF16)           # src-fold
    dis_pad = np.zeros(n_pad, np.float32)
    dis_pad[:N] = dis

    in_maps = []
    for c in range(NCORES):
        idx_flat = np.zeros(ncols * P, np.int16)
        src_flat = np.zeros(ncols * P, np.int64)
        dlc_flat = np.zeros(ncols * P, np.int64)
        val_flat = np.zeros(ncols * P, bool)
        for g_ops, g_blocks in ops_plan:
            for (t, chunk_off, nchunks) in g_ops:
                p0 = chunk_off * P
                for (j, segs) in g_blocks:
                    for (tt, _, k0, nk) in segs:
                        if tt != t:
                            continue
                        s = starts[((c * nblk + j) * NTAB) + t]
                        e = starts[((c * nblk + j) * NTAB) + t + 1]
                        n = e - s
                        sl = slice(p0 + k0 * P, p0 + k0 * P + n)
                        idx_flat[sl] = (row_s[s:e] % T).astype(np.int16)
                        src_flat[sl] = row_s[s:e]
                        dlc_flat[sl] = col_s[s:e] - (c * nblk + j) * P
                        val_flat[sl] = True
        idx_arr = np.tile(idx_flat.reshape(-1, 16).T, (8, 1)).astype(np.int16)
        xg = np.zeros((ncols * P, F), dtype=BF16)
        xg[val_flat] = xn[src_flat[val_flat]]
        im = {
            "xg_in": xg,
            "idx_in": np.ascontiguousarray(idx_arr),
            "disb": np.ascontiguousarray(
                dis_pad[c*slice_rows:(c+1)*slice_rows].reshape(nblk, P).T),
            "disr": np.ascontiguousarray(
                dis_pad[c*slice_rows:(c+1)*slice_rows].reshape(1, slice_rows)),
        }
        if S_MODE == "stream":
            # sf[p, c*128+q] = 1 if dlc[c*128+p] == q and valid
            oh = (dlc_flat.reshape(ncols, P)[:, :, None] ==
                  np.arange(P)[None, None, :])
            oh &= val_flat.reshape(ncols, P)[:, :, None]
            sf = np.ascontiguousarray(
                oh.transpose(1, 0, 2).reshape(P, ncols * P).astype(BF16))
            im["sf_in"] = sf
        else:
            dlc_m = np.where(val_flat, dlc_flat, -1).astype(np.float32)
            im["mdlc"] = np.ascontiguousarray(dlc_m.reshape(ncols, P).T)
        in_maps.append(im)
    return ops_plan, in_maps, ncols, idxcols, ch_max


LAST_RESULTS = None
LAST_NC = None
LAST_IN_MAPS = None


def kernel(x, edge_index, W0, b0, W1, b1, W2, b2, Wp, bp):
    global LAST_RESULTS, LAST_NC, LAST_IN_MAPS
    from concourse.bass_utils import run_bass_kernel_spmd

    x = np.asarray(x, dtype=np.float32)
    edge_index = np.asarray(edge_index)
    N = x.shape[0]
    nblk = int(np.ceil(N / (NCORES * P)))
    n_pad = NCORES * nblk * P
    slice_rows = nblk * P

    ops_plan, in_maps, ncols, idxcols, ch_max = _prep(x, edge_index, n_pad, nblk)

    nc = _build_bass(
        nblk, n_pad, ncols, idxcols, ch_max, ops_plan,
        [np.asarray(W0), np.asarray(W1), np.asarray(W2)],
        [np.asarray(b0), np.asarray(b1), np.asarray(b2)],
        np.asarray(Wp), np.asarray(bp),
    )

    res = run_bass_kernel_spmd(nc, in_maps, list(range(NCORES)))
    LAST_RESULTS = res
    LAST_NC = nc
    LAST_IN_MAPS = in_maps

    out = np.zeros((n_pad, 2), np.float32)
    for c in range(NCORES):
        yT = res.results[c]["y_out"]          # [2, slice_rows]
        out[c*slice_rows:(c+1)*slice_rows] = yT.T
    return out[:N]


# revision 22
# speedup vs baseline: 1.0361x; 1.0014x over previous
import os
import sys
sys.path.insert(0, '/opt/trn_rl_repo')
import numpy as np
import ml_dtypes

BF16 = ml_dtypes.bfloat16

P = 128
NCORES = 8
F = 128
NTAB = 4          # gather tables (int16 index range: rows per table <= 32767)
G_BLK = int(os.environ.get("G_BLK", "4"))       # dest blocks per gather group
S_MODE = os.environ.get("S_MODE", "build")      # build | stream


def _build_bass(nblk, n_pad, ncols, idxcols, ch_max, ops_plan, weights, biases, Wp, bp):
    """SPMD Bass program. Identical trace on all cores; per-core data via
    ExternalInputs.

    The GCN normalization is folded: gather tables hold h*dis[node] (source
    side); the dest-side dis is applied inside each layer's bias/relu and,
    for the final conv, once in the epilogue. The scatter one-hot S is
    therefore pure 0/1.

    ops_plan: list of groups; each group is (ops, blocks) with
        op = (t, chunk_off, nchunks)            # one dma_gather per src range
        block = (j, [(t, chunk_off, k0, nk)])   # matmul chunks per block
    """
    from concourse import bass, bacc, mybir
    import concourse.tile as tile

    slice_rows = nblk * P
    T = n_pad // NTAB

    nc = bacc.Bacc(num_devices=NCORES, num_swdge_queues=4)

    bf = mybir.dt.bfloat16
    f32 = mybir.dt.float32

    xg_in = nc.declare_dram_parameter("xg_in", [ncols * P, F], bf, isOutput=False)
    xloc_in = nc.declare_dram_parameter("xloc", [slice_rows, F], bf, isOutput=False)
    idx_in = nc.declare_dram_parameter("idx_in", [P, idxcols], mybir.dt.int16, isOutput=False)
    disb_in = nc.declare_dram_parameter("disb", [P, nblk], f32, isOutput=False)
    disr_in = nc.declare_dram_parameter("disr", [1, slice_rows], f32, isOutput=False)
    if S_MODE == "stream":
        sf_in = nc.declare_dram_parameter("sf_in", [P, ncols * P], bf, isOutput=False)
    else:
        mdlc_in = nc.declare_dram_parameter("mdlc", [P, ncols], f32, isOutput=False)
    y_out = nc.declare_dram_parameter("y_out", [2, slice_rows], f32, isOutput=True)

    # internal DRAM
    h_slice = [nc.dram_tensor(f"h_slice{i}", [slice_rows, F], bf) for i in range(3)]
    v_full = [nc.dram_tensor(f"v_full{i}", [n_pad, F], bf, addr_space="Shared")
              for i in range(3)]

    # inline constants (same on every core)
    W_d = [nc.inline_tensor(np.ascontiguousarray(w.astype(BF16)), name=f"W{i}")
           for i, w in enumerate(weights)]
    B_d = [nc.inline_tensor(np.broadcast_to(b.astype(np.float32), (P, F)).copy(), name=f"B{i}")
           for i, b in enumerate(biases)]
    Wp_d = [nc.inline_tensor(np.ascontiguousarray(Wp[i*F:(i+1)*F, :].astype(BF16)), name=f"Wp{i}")
            for i in range(3)]
    bpd = float(bp[0] - bp[1])
    ident_d = nc.inline_tensor(np.eye(P, dtype=np.float32).astype(BF16), name="ident_c")
    if S_MODE == "build":
        iota_np = np.broadcast_to(np.arange(P, dtype=np.float32), (P, P)).astype(BF16)
        iota_d = nc.inline_tensor(np.ascontiguousarray(iota_np), name="iota_c")

    AF = mybir.ActivationFunctionType
    ALU = mybir.AluOpType
    rg = [list(range(NCORES))]

    with tile.TileContext(nc) as tc:
        with (
            tc.tile_pool(name="const", bufs=1) as cpool,
            tc.tile_pool(name="msg", bufs=6) as mpool,
            tc.tile_pool(name="sS", bufs=6) as spool,
            tc.tile_pool(name="gts", bufs=4) as gpool,
            tc.tile_pool(name="hout", bufs=4) as hpool,
            tc.tile_pool(name="hloc", bufs=4) as lpool,
            tc.tile_pool(name="epi", bufs=1) as epool,
            tc.tile_pool(name="psum", bufs=3, space="PSUM") as psum,
            tc.tile_pool(name="psum2", bufs=2, space="PSUM") as psum2,
        ):
            W_t, B_t, Wp_t = [], [], []
            for i in range(3):
                wt = cpool.tile([P, F], bf, tag=f"w{i}")
                nc.sync.dma_start(out=wt[:], in_=W_d[i][:, :])
                W_t.append(wt)
                bt = cpool.tile([P, F], f32, tag=f"b{i}")
                nc.sync.dma_start(out=bt[:], in_=B_d[i][:, :])
                B_t.append(bt)
                wpt = cpool.tile([P, 2], bf, tag=f"wp{i}")
                nc.sync.dma_start(out=wpt[:], in_=Wp_d[i][:, :])
                Wp_t.append(wpt)

            idx_t = cpool.tile([P, idxcols], mybir.dt.int16)
            nc.sync.dma_start(out=idx_t[:], in_=idx_in[:, :])
            disb_t = cpool.tile([P, nblk], f32)
            nc.sync.dma_start(out=disb_t[:], in_=disb_in[:, :])
            if S_MODE == "build":
                iota_t = cpool.tile([P, P], bf)
                nc.sync.dma_start(out=iota_t[:], in_=iota_d[:, :])
                mdlc_t = cpool.tile([P, ncols], f32)
                nc.sync.dma_start(out=mdlc_t[:], in_=mdlc_in[:, :])

            ident_t = cpool.tile([P, P], bf, tag="ident")
            nc.sync.dma_start(out=ident_t[:], in_=ident_d[:, :])
            yT_acc = cpool.tile([2, slice_rows], f32)
            nc.vector.memset(yT_acc[:], 0.0)

            qrot = 0
            # ---- 4 propagations ----
            for i in range(4):
                src = v_full[i-1] if i >= 1 else None
                for g_ops, g_blocks in ops_plan:
                    msgs = {}
                    ss = {}
                    for (t, chunk_off, nchunks) in g_ops:
                        mt = mpool.tile([P, ch_max, F], bf, tag="msg")
                        n = nchunks * P
                        if i == 0:
                            nc.sync.dma_start(
                                out=mt[:, :nchunks, :],
                                in_=xg_in[chunk_off*P:(chunk_off+nchunks)*P, :]
                                    .rearrange("(c p) f -> p c f", p=P))
                        else:
                            nc.gpsimd.dma_gather(
                                mt[:, :nchunks, :],
                                src[t*T:(t+1)*T, :],
                                idx_t[:, 8*chunk_off: 8*chunk_off + 8*nchunks],
                                n, n, F,
                                single_packet=False,
                                queue_num=qrot,
                            )
                            qrot = (qrot + 1) % 4
                        msgs[t] = (mt, chunk_off)
                        if S_MODE == "stream":
                            st = spool.tile([P, ch_max * P], bf, tag="S")
                            nc.sync.dma_start(
                                out=st[:, :nchunks*P],
                                in_=sf_in[:, chunk_off*P:(chunk_off+nchunks)*P])
                            ss[t] = st
                    for (j, segs) in g_blocks:
                        nk_tot = sum(nk for (_, _, _, nk) in segs) + 1
                        gt = psum.tile([P, P], f32, tag="gt", space="PSUM")
                        hl = lpool.tile([P, F], bf, tag="hl")
                        if i == 0:
                            nc.sync.dma_start(out=hl[:], in_=xloc_in[j*P:(j+1)*P, :])
                        else:
                            nc.sync.dma_start(out=hl[:], in_=h_slice[i-1][j*P:(j+1)*P, :])
                        nc.tensor.matmul(out=gt[:], lhsT=hl[:], rhs=ident_t[:],
                                         start=True, stop=(nk_tot == 1))
                        kk = 1
                        for (t, chunk_off, k0, nk) in segs:
                            mt, op_off = msgs[t]
                            for k in range(nk):
                                col = chunk_off + k0 + k
                                cl = col - op_off
                                if S_MODE == "stream":
                                    S = ss[t][:, cl*P:(cl+1)*P]
                                else:
                                    St = spool.tile([P, P], bf, tag="S")
                                    nc.vector.tensor_scalar(
                                        out=St[:], in0=iota_t[:],
                                        scalar1=mdlc_t[:, col:col+1], scalar2=None,
                                        op0=ALU.is_equal)
                                    S = St[:]
                                nc.tensor.matmul(
                                    out=gt[:], lhsT=mt[:, cl, :], rhs=S,
                                    start=(kk == 0), stop=(kk == nk_tot - 1))
                                kk += 1
                        gts = gpool.tile([P, P], bf, tag="gts")
                        nc.scalar.copy(out=gts[:], in_=gt[:])
                        if i < 3:
                            hp = psum2.tile([P, P], f32, tag="hp", space="PSUM")
                            nc.tensor.matmul(out=hp[:], lhsT=gts[:], rhs=W_t[i][:],
                                             start=True, stop=True)
                            hb = hpool.tile([P, P], f32, tag="hb")
                            # h = relu(dis_dst*hp + B); table row = h*dis (src fold)
                            nc.vector.scalar_tensor_tensor(
                                out=hb[:], in0=hp[:], scalar=disb_t[:, j:j+1],
                                in1=B_t[i][:], op0=ALU.mult, op1=ALU.add)
                            hb2 = hpool.tile([P, P], bf, tag="hb2")
                            nc.vector.tensor_scalar(
                                out=hb2[:], in0=hb[:],
                                scalar1=0.0, op0=ALU.max,
                                scalar2=disb_t[:, j:j+1], op1=ALU.mult)
                            nc.sync.dma_start(out=h_slice[i][j*P:(j+1)*P, :], in_=hb2[:])
                        if i >= 1:
                            yp = psum2.tile([2, P], f32, tag="yp", space="PSUM")
                            nc.tensor.matmul(out=yp[:], lhsT=Wp_t[i-1][:], rhs=gts[:],
                                             start=True, stop=True)
                            nc.vector.tensor_tensor(
                                out=yT_acc[:, j*P:(j+1)*P],
                                in0=yT_acc[:, j*P:(j+1)*P], in1=yp[:], op=ALU.add)
                if i < 3:
                    nc.gpsimd.collective_compute(
                        "AllGather", ALU.bypass, replica_groups=rg,
                        ins=[h_slice[i][:].opt()], outs=[v_full[i][:].opt()],
                    )

            # ---- epilogue: d = dis*(y0-y1); p0 = sigmoid(d + bp0-bp1) ----
            nchunk_e = 8
            cw = slice_rows // nchunk_e
            for ce in range(nchunk_e):
                cs = ce * cw
                t1 = epool.tile([1, cw], f32, tag="t1")
                nc.sync.dma_start(out=t1[:], in_=yT_acc[1:2, cs:cs+cw])
                dr = epool.tile([1, cw], f32, tag="dr")
                nc.sync.dma_start(out=dr[:], in_=disr_in[0:1, cs:cs+cw])
                dif = epool.tile([1, cw], f32, tag="dif")
                nc.vector.tensor_tensor(out=dif[:], in0=yT_acc[0:1, cs:cs+cw],
                                        in1=t1[:], op=ALU.subtract)
                nc.vector.tensor_tensor(out=dif[:], in0=dif[:], in1=dr[:], op=ALU.mult)
                nc.vector.tensor_scalar(out=dif[:], in0=dif[:],
                                        scalar1=bpd, scalar2=None, op0=ALU.add)
                sig = epool.tile([1, cw], f32, tag="sig")
                nc.scalar.activation(out=sig[:], in_=dif[:], func=AF.Sigmoid)
                om = epool.tile([1, cw], f32, tag="om")
                nc.vector.tensor_scalar(out=om[:], in0=sig[:],
                                        scalar1=-1.0, op0=ALU.mult,
                                        scalar2=1.0, op1=ALU.add)
                nc.sync.dma_start(out=y_out[0:1, cs:cs+cw], in_=sig[:])
                nc.sync.dma_start(out=y_out[1:2, cs:cs+cw], in_=om[:])

    nc.compile()
    return nc


def _prep(x, edge_index, n_pad, nblk):
    """Host-side prep: fold norm into tables, bucket edges by
    (dest block, src range), pad buckets to the max across cores, build
    gather indices, prop-0 pre-gathered messages, and (for S_MODE=stream)
    the streamed one-hot scatter matrices."""
    N = x.shape[0]
    T = n_pad // NTAB
    slice_rows = nblk * P
    nblk_tot = NCORES * nblk

    row = edge_index[0].astype(np.int64)
    col = edge_index[1].astype(np.int64)
    deg = (np.bincount(col, minlength=N) + 1).astype(np.float32)   # + self-loop
    dis = (1.0 / np.sqrt(deg)).astype(np.float32)

    gblk = col // P
    trng = row // T
    key = gblk * NTAB + trng
    order = np.argsort(key, kind='stable')
    row_s = row[order].astype(np.int32)
    col_s = col[order].astype(np.int32)

    cnt = np.bincount(key, minlength=nblk_tot * NTAB).reshape(NCORES, nblk, NTAB)
    starts = np.zeros(nblk_tot * NTAB + 1, np.int64)
    np.cumsum(cnt.reshape(-1), out=starts[1:])

    K_jt = np.ceil(cnt.max(axis=0) / P).astype(np.int64)   # [nblk, NTAB]

    ops_plan = []
    colp = 0
    ch_max = 0
    for g in range(0, nblk, G_BLK):
        blocks = list(range(g, min(g + G_BLK, nblk)))
        g_ops = []
        segs_per_block = {j: [] for j in blocks}
        for t in range(NTAB):
            nchunks = int(sum(K_jt[j, t] for j in blocks))
            if nchunks == 0:
                continue
            g_ops.append((t, colp, nchunks))
            ch_max = max(ch_max, nchunks)
            k0 = 0
            for j in blocks:
                nk = int(K_jt[j, t])
                if nk:
                    segs_per_block[j].append((t, colp, k0, nk))
                k0 += nk
            colp += nchunks
        g_blocks = [(j, segs_per_block[j]) for j in blocks]
        ops_plan.append((g_ops, g_blocks))
    ncols = colp
    idxcols = 8 * ncols

    xn = (x * dis[:, None]).astype(BF16)           # src-fold
    dis_pad = np.zeros(n_pad, np.float32)
    dis_pad[:N] = dis

    in_maps = []
    for c in range(NCORES):
        idx_flat = np.zeros(ncols * P, np.int16)
        src_flat = np.zeros(ncols * P, np.int64)
        dlc_flat = np.zeros(ncols * P, np.int64)
        val_flat = np.zeros(ncols * P, bool)
        for g_ops, g_blocks in ops_plan:
            for (t, chunk_off, nchunks) in g_ops:
                p0 = chunk_off * P
                for (j, segs) in g_blocks:
                    for (tt, _, k0, nk) in segs:
                        if tt != t:
                            continue
                        s = starts[((c * nblk + j) * NTAB) + t]
                        e = starts[((c * nblk + j) * NTAB) + t + 1]
                        n = e - s
                        sl = slice(p0 + k0 * P, p0 + k0 * P + n)
                        idx_flat[sl] = (row_s[s:e] % T).astype(np.int16)
                        src_flat[sl] = row_s[s:e]
                        dlc_flat[sl] = col_s[s:e] - (c * nblk + j) * P
                        val_flat[sl] = True
        idx_arr = np.tile(idx_flat.reshape(-1, 16).T, (8, 1)).astype(np.int16)
        xg = np.zeros((ncols * P, F), dtype=BF16)
        xg[val_flat] = xn[src_flat[val_flat]]
        im = {
            "xg_in": xg,
            "xloc": np.ascontiguousarray(xn[c*slice_rows:(c+1)*slice_rows])
                if (c+1)*slice_rows <= N else
                np.ascontiguousarray(np.vstack([
                    xn[c*slice_rows:N],
                    np.zeros(((c+1)*slice_rows - N, F), dtype=BF16)])),
            "idx_in": np.ascontiguousarray(idx_arr),
            "disb": np.ascontiguousarray(
                dis_pad[c*slice_rows:(c+1)*slice_rows].reshape(nblk, P).T),
            "disr": np.ascontiguousarray(
                dis_pad[c*slice_rows:(c+1)*slice_rows].reshape(1, slice_rows)),
        }
        if S_MODE == "stream":
            # sf[p, c*128+q] = 1 if dlc[c*128+p] == q and valid
            oh = (dlc_flat.reshape(ncols, P)[:, :, None] ==
                  np.arange(P)[None, None, :])
            oh &= val_flat.reshape(ncols, P)[:, :, None]
            sf = np.ascontiguousarray(
                oh.transpose(1, 0, 2).reshape(P, ncols * P).astype(BF16))
            im["sf_in"] = sf
        else:
            dlc_m = np.where(val_flat, dlc_flat, -1).astype(np.float32)
            im["mdlc"] = np.ascontiguousarray(dlc_m.reshape(ncols, P).T)
        in_maps.append(im)
    return ops_plan, in_maps, ncols, idxcols, ch_max


LAST_RESULTS = None
LAST_NC = None
LAST_IN_MAPS = None


def kernel(x, edge_index, W0, b0, W1, b1, W2, b2, Wp, bp):
    global LAST_RESULTS, LAST_NC, LAST_IN_MAPS
    from concourse.bass_utils import run_bass_kernel_spmd

    x = np.asarray(x, dtype=np.float32)
    edge_index = np.asarray(edge_index)
    N = x.shape[0]
    nblk = int(np.ceil(N / (NCORES * P)))
    n_pad = NCORES * nblk * P
    slice_rows = nblk * P

    ops_plan, in_maps, ncols, idxcols, ch_max = _prep(x, edge_index, n_pad, nblk)

    nc = _build_bass(
        nblk, n_pad, ncols, idxcols, ch_max, ops_plan,
        [np.asarray(W0), np.asarray(W1), np.asarray(W2)],
        [np.asarray(b0), np.asarray(b1), np.asarray(b2)],
        np.asarray(Wp), np.asarray(bp),
    )

    res = run_bass_kernel_spmd(nc, in_maps, list(range(NCORES)))
    LAST_RESULTS = res
    LAST_NC = nc
    LAST_IN_MAPS = in_maps

    out = np.zeros((n_pad, 2), np.float32)
    for c in range(NCORES):
        yT = res.results[c]["y_out"]          # [2, slice_rows]
        out[c*slice_rows:(c+1)*slice_rows] = yT.T
    return out[:N]
